# revision 50
# baseline (speedup 1.0000x reference)
"""Trainium2 Bass kernel for CrossModalAttention (v2).

Reference computation (B=1, C=64, N=8192 voxels): two cross-attention
directions (CT queries over MRI keys/values and vice versa), each with an
8192x8192 attention matrix, fused output projection.

Sharding: each of the 8 cores owns 1024 query voxels for BOTH directions,
computes K/V over the full sequence locally, and produces its own
(64, 1024) slice of the output. No collectives.

Design (196us v1 baseline -> 127us):
  * K-side bias dropped (a per-query additive constant cancels in softmax),
    so the score contraction is exactly 64 channels -> both score matmuls of
    a pass run CONCURRENTLY as 64x128 row tiles (tile_position (0,0)/(64,0)),
    reading features + a duplicated qq from opposite SBUF partition halves.
    Features are stored host-side as (128, 4096): top half = key chunks
    0..31 channels, bottom half = chunks 32..63.
  * exp is split across ScalarE (table exp, 18/32 passes) and VectorE (a
    custom DVE op registered at import, evaluating
    exp(x) ~= (1 + x(c1 + x(c2 + x c3)))^4 in one 8-ALU-stage pass, max rel
    err 5.5e-4 on |x|<=1.5). ScalarE alone (1 elem/lane/cycle @1.2 GHz)
    would cost ~109us for the 16.8M scores and dominate the kernel; the
    split makes exp and PE roughly co-critical.
  * AV flipped to out=(c, i): att^T accumulates into (65, 512) PSUM banks
    via lhsT=vT (key-major V chunks in fp8e4m3, 65th column = ones
    accumulating the softmax denominator), rhs = fp8 exp tiles, with
    perf_mode=DoubleRow: each 64-row tile contracts a chunk PAIR (2 fp8
    weights/cell, 2 MACs/cell/cycle) -- ~2x AV throughput. AV stays in the
    64x128 tiling mode of the scores (mode switches drain the PE) and is
    emitted in blocks of 4 passes to amortize the normal<->DoubleRow
    perf-mode switch. Two partial banks (key halves) are summed during
    normalization.
  * V bias folded into the output-projection bias on the host; softmax
    division via approximate-reciprocal (DVE) of the denominator row +
    GPSIMD partition_broadcast (GPSIMD must run ONLY this op -- mixing Q7
    libraries forces ~6us LIBRARY_RELOAD stalls).
  * Output projection = two accumulating K=64 matmuls per query half.
  * All small inputs packed into one DRAM tensor (DMA dispatch costs ~1us
    per dma_start on the queue); V projections and the second qq projection
    are interleaved into attention passes so the PE never idles behind
    them; normalization is evacuated through SBUF (ScalarE+DVE copies) so
    the next stage can recycle the AV banks immediately.

Precision: scores fp16 operands, fp32 PSUM accumulation everywhere, et/vT
fp8e4m3; measured end-to-end rel err 3.2e-3 (tolerance 2e-2).
"""

from contextlib import ExitStack

import numpy as np

import concourse.bass as bass
import concourse.mybir as mybir
import concourse.tile as tile
from concourse import bacc
from concourse.bass_utils import run_bass_kernel_spmd

F32 = mybir.dt.float32
F16 = mybir.dt.float16
F8 = mybir.dt.float8e4
# fp8 DoubleRow AV: et/vT stored fp8e4m3; each AV matmul contracts a chunk
# PAIR (2 fp8 weights per PE cell, 2 MACs/cell/cycle) -- ~2x AV throughput
# for ~3e-3 extra relative error (tolerance is 2e-2)
FP8_AV = True
ETDT = F8 if FP8_AV else F16
C = 64          # channels
N = 8192        # voxels (8*32*32)
NCORES = 8
NQ = N // NCORES      # 1024 queries per core
IH = 512              # query block (PSUM bank width in f32)
NIH = NQ // IH        # 2
JCH = 128             # key chunk
NJ = N // JCH         # 64 chunks; row-tile pass c handles chunks c and 32+c
NP = NJ // 2          # 32 row-tiled passes
W = C + 1             # 65: V channels + denominator-ones column
HF = N // 2           # 4096: feature columns per partition half
NFS = 2               # feature DMA subtiles per modality (4 KB lines)
FSW = HF // NFS       # 2048
SMW = 2 * NQ + 6 * C + 8   # packed small-inputs tensor width
LAG = 3               # exp groups the AV matmuls trail behind
AVBLK = 16            # AV passes emitted per perf-mode switch block
# exp(x) ~= (1 + x(c1 + x(c2 + x c3)))^4 minimax-fit on [-1.5, 1.5]
EC1, EC2, EC3 = 0.2500669, 0.03155202, 0.0025770527
# score-pass exp engine split: ScalarE (1.2 GHz) vs DVE poly (0.96 GHz)
SCALAR_SHARE = 17     # of 32 passes go to ScalarE


def _register_exp_op():
    """Register the EXP_POLY4_ANT custom DVE op (runtime equivalent of the
    documented 'append a DveOp to dve_ops.OPS' extension path)."""
    import concourse.dve_ops as dve_ops
    from concourse.dve_spec import Spec, Src0, C0, C1, C2, One, sq, lower
    from concourse.dve_uop import DveOpSpec

    if "EXP_POLY4_ANT" in dve_ops._SUB_OPCODE_FOR_NAME:
        return next(o for o in dve_ops.OPS if o.name == "EXP_POLY4_ANT")

    def _ref(in0, in1, s0, s1, imm2):
        x = in0.astype(np.float32)
        p = 1.0 + x * (s0 + x * (s1 + x * imm2))
        p2 = p * p
        return p2 * p2

    spec = Spec(
        body=sq(sq(One + Src0 * (C0 + Src0 * (C1 + Src0 * C2)))),
        reference=_ref,
    )
    opcode = dve_ops._CUSTOM_DVE_ROW_BASE + len(dve_ops.OPS)
    shas = {}
    for ver in ("v3", "v4"):
        s = DveOpSpec(
            name="EXP_POLY4_ANT", opcode=opcode, uops=lower(spec, ver=ver),
            rd1_en=False,
        )
        shas[ver] = s.sha(ver)
    op = dve_ops.DveOp("EXP_POLY4_ANT", spec, subdim=False, uops_sha=shas)
    dve_ops.OPS.append(op)
    dve_ops.CUSTOM_DVE_SPECS[op.name] = op.spec
    dve_ops._SUB_OPCODE_FOR_NAME[op.name] = opcode
    return op


EXP_OP = _register_exp_op()


def _emit_attention(nc, pools, feat, qq, vT, ih, d, att_out, aux=None,
                    tail_engine=None):
    """One (direction, query-half) attention stage: 32 row-tiled score
    passes, engine-split exp, row-tiled AV accumulation, normalization.
    `aux` (pass_idx -> None) interleaves auxiliary emission (vproj batches
    for the next modality) between score passes."""
    sp, avp, etp, rp, tp = (
        pools["sp"], pools["avp"], pools["etp"], pools["rp"], pools["tp"],
    )
    qlo, qhi = qq[0:64, IH * ih : IH * (ih + 1)], qq[64:128, IH * ih : IH * (ih + 1)]

    avA = avp.tile([JCH, IH], F32, tag="av", name=f"avA{d}{ih}")
    avB = avp.tile([JCH, IH], F32, tag="av", name=f"avB{d}{ih}")

    def emit_av(c, et):
        if FP8_AV:
            # one DoubleRow matmul per 64-key row tile covers the whole
            # chunk pair (c, NP+c): lhsT [64, 2, 65] (pair stride NP*W fp8
            # bytes, 16-aligned), rhs [64, 2, 512]
            for p0, av in ((0, avA), (64, avB)):
                nc.tensor.matmul(
                    av[0:W, :],
                    lhsT=vT[p0 : p0 + 64, :].rearrange(
                        "p (g kw) -> p g kw", g=2
                    )[:, :, W * c : W * (c + 1)],
                    rhs=et[p0 : p0 + 64, :].rearrange("p (g i) -> p g i", g=2),
                    start=(c == 0),
                    stop=(c == NP - 1),
                    skip_group_check=True,
                    perf_mode=mybir.MatmulPerfMode.DoubleRow,
                )
            return
        for idx, j in enumerate((c, NP + c)):
            sl = slice(IH * idx, IH * (idx + 1))
            nc.tensor.matmul(
                avA[0:W, :],
                lhsT=vT[0:64, W * j : W * (j + 1)],
                rhs=et[0:64, sl],
                start=(c == 0 and idx == 0),
                stop=(c == NP - 1 and idx == 1),
                skip_group_check=True,
            )
            nc.tensor.matmul(
                avB[0:W, :],
                lhsT=vT[64:128, W * j : W * (j + 1)],
                rhs=et[64:128, sl],
                start=(c == 0 and idx == 0),
                stop=(c == NP - 1 and idx == 1),
                skip_group_check=True,
            )

    pending = []
    for c in range(NP):
        if aux is not None:
            aux(c)
        ps = sp.tile([JCH, 2 * IH], F32, tag="ps", name=f"ps{d}{ih}{c}")
        nc.tensor.matmul(
            ps[:, 0:IH],
            lhsT=feat[0:64, JCH * c : JCH * (c + 1)],
            rhs=qlo, start=True, stop=True,
        )
        nc.tensor.matmul(
            ps[:, IH : 2 * IH],
            lhsT=feat[64:128, JCH * c : JCH * (c + 1)],
            rhs=qhi, start=True, stop=True,
        )
        et = etp.tile([JCH, 2 * IH], ETDT, tag="et", name=f"et{d}{ih}{c}")
        if (c * SCALAR_SHARE) % NP < SCALAR_SHARE:
            nc.scalar.activation(et[:], ps[:], mybir.ActivationFunctionType.Exp)
        else:
            nc.vector._custom_dve(
                EXP_OP, out=et[:], in0=ps[:], s0=EC1, s1=EC2, imm2=EC3
            )
        pending.append((c, et))
        # flush AV in blocks of AVBLK passes: each normal<->DoubleRow
        # perf-mode change reconfigures the PE, so batching the AV matmuls
        # amortizes the switch over 2*AVBLK matmuls
        if len(pending) >= LAG + AVBLK:
            for _ in range(AVBLK):
                emit_av(*pending.pop(0))
    for args in pending:
        emit_av(*args)

    # normalization: att = (A + B)[0:64] * bcast(1 / (denA + denB)).
    # ScalarE (adjacent to PSUM) evacuates both AV banks to SBUF right away
    # so the next stage's accumulation can recycle them; everything after
    # runs off the critical PE path (DVE + a GPSIMD partition-broadcast --
    # a PE ones-matmul here would cost a tiling-mode-switch drain and stall
    # the in-order PE queue behind the DVE reciprocal).
    sa = tp.tile([W, IH], F32, tag="sa", name=f"sa{d}{ih}")
    nc.scalar.copy(sa[:], avA[0:W, :])
    sb = tp.tile([W, IH], F32, tag="sb", name=f"sb{d}{ih}")
    nc.vector.tensor_copy(sb[:], avB[0:W, :])
    den = rp.tile([1, IH], F32, tag="den", name=f"den{d}{ih}")
    nc.vector.tensor_tensor(
        den[:], sa[C : C + 1, :], sb[C : C + 1, :], op=mybir.AluOpType.add
    )
    r = rp.tile([1, IH], F32, tag="r", name=f"r{d}{ih}")
    nc.vector.reciprocal_approx_fast(r[:], den[:])
    # NOTE: keep GPSIMD running ONLY partition_broadcast -- mixing in
    # tensor ops forces a Q7 LIBRARY_RELOAD (~6us stall) at every switch
    rb = rp.tile([C, IH], F32, tag="rb", name=f"rb{d}{ih}")
    nc.gpsimd.partition_broadcast(rb[:], r[:], channels=C)
    t = tp.tile([C, IH], F16, tag="t", name=f"t{d}{ih}")
    nc.vector.tensor_tensor(t[:], sa[0:C, :], sb[0:C, :], op=mybir.AluOpType.add)
    nc.vector.tensor_tensor(att_out[:], t[:], rb[:], op=mybir.AluOpType.mult)


def _emit_vproj_batch(nc, pools, feat, wv_sb, m, vT, b):
    """One batch of 8 chunk pairs of the V projection, key-major: 64x128
    row-tiled matmuls, then fp32->fp16 copies into the strided vT store
    (65-wide slots; the 65th ones-column is pre-memset)."""
    sp = pools["sp"]
    ps = sp.tile([JCH, 2 * IH], F32, tag="ps", name=f"pv{m}{b}")
    for k in range(8):
        c = 8 * b + k
        nc.tensor.matmul(
            ps[:, 64 * k : 64 * (k + 1)],
            lhsT=feat[0:64, JCH * c : JCH * (c + 1)],
            rhs=wv_sb[0:64, 64 * m : 64 * (m + 1)],
            start=True, stop=True,
        )
        nc.tensor.matmul(
            ps[:, IH + 64 * k : IH + 64 * (k + 1)],
            lhsT=feat[64:128, JCH * c : JCH * (c + 1)],
            rhs=wv_sb[64:128, 64 * m : 64 * (m + 1)],
            start=True, stop=True,
        )
    # both copies on ScalarE: the DVE is the binding engine in steady state
    # (its share of exp + normalization), ScalarE has the slack
    v3 = vT[:].rearrange("p (k w) -> p k w", w=W)
    s3lo = ps[:, 0:IH].rearrange("p (k w) -> p k w", w=64)
    s3hi = ps[:, IH : 2 * IH].rearrange("p (k w) -> p k w", w=64)
    nc.scalar.copy(v3[:, 8 * b : 8 * (b + 1), 0:64], s3lo)
    nc.scalar.copy(v3[:, NP + 8 * b : NP + 8 * (b + 1), 0:64], s3hi)


def _emit_qq(nc, pools, wqq_sb, qsrc, d, qq):
    """qq duplicated into both partition halves via 2x column tiling."""
    sp = pools["sp"]
    for ih in range(NIH):
        ps = sp.tile([JCH, 2 * IH], F32, tag="ps", name=f"pqq{d}{ih}")
        for half in range(2):
            nc.tensor.matmul(
                ps[64 * half : 64 * (half + 1), 0:IH],
                lhsT=wqq_sb[:, 64 * d : 64 * (d + 1)],
                rhs=qsrc[:, IH * ih : IH * (ih + 1)],
                start=True, stop=True,
                tile_position=(0, 64 * half),
            )
        nc.vector.tensor_copy(qq[:, IH * ih : IH * (ih + 1)], ps[:, 0:IH])


def _build_program(ctx, tc, ct, mri, smalls, out):
    nc = tc.nc
    wpool = ctx.enter_context(tc.tile_pool(name="wpool", bufs=1))
    featp = ctx.enter_context(tc.tile_pool(name="feat", bufs=2 * NFS))
    pools = {
        "sp": ctx.enter_context(tc.tile_pool(name="spsum", bufs=3, space="PSUM")),
        "avp": ctx.enter_context(tc.tile_pool(name="avp", bufs=2, space="PSUM")),
        "etp": ctx.enter_context(tc.tile_pool(name="etp", bufs=22)),
        "rp": ctx.enter_context(tc.tile_pool(name="rp", bufs=2)),
        "tp": ctx.enter_context(tc.tile_pool(name="tp", bufs=2)),
    }
    vtp = ctx.enter_context(tc.tile_pool(name="vtp", bufs=2))
    qqp = ctx.enter_context(tc.tile_pool(name="qqp", bufs=2))
    attp = ctx.enter_context(tc.tile_pool(name="attp", bufs=4))
    op_ = ctx.enter_context(tc.tile_pool(name="outp", bufs=2))

    # DMA dispatch costs ~1us each on the queue; all small inputs arrive
    # packed in ONE host-prepared tensor (single dispatch), features in 3
    fs_mri = wpool.tile([JCH, HF], F16, name="fs_mri")
    fs_ct = wpool.tile([JCH, HF], F16, name="fs_ct")
    sm = wpool.tile([JCH, SMW], F16, name="sm")
    nc.sync.dma_start(sm[:], smalls[:])
    nc.sync.dma_start(fs_mri[:, 0:1024], mri[:, 0:1024])
    nc.sync.dma_start(fs_mri[:, 1024:HF], mri[:, 1024:HF])
    nc.sync.dma_start(fs_ct[:], ct[:])
    qsc = sm[0:W, 0:NQ]
    qsm = sm[0:W, NQ : 2 * NQ]
    wqq_sb = sm[0:W, 2 * NQ : 2 * NQ + 2 * C]
    wv_sb = sm[0:JCH, 2 * NQ + 2 * C : 2 * NQ + 4 * C]
    woT_sb = sm[0:C, 2 * NQ + 4 * C : 2 * NQ + 6 * C]
    bo_sb = wpool.tile([C, 1], F32, name="bo_sb")
    nc.vector.tensor_copy(bo_sb[:], sm[0:C, 2 * NQ + 6 * C : 2 * NQ + 6 * C + 1])

    vT_mri = vtp.tile([JCH, NJ * W], ETDT, tag="vt", name="vT_mri")
    vT_ct = vtp.tile([JCH, NJ * W], ETDT, tag="vt", name="vT_ct")
    for vt in (vT_mri, vT_ct):
        nc.vector.memset(
            vt[:].rearrange("p (k w) -> p k w", w=W)[:, :, C : C + 1], 1.0
        )

    qq0 = qqp.tile([JCH, NQ], F16, tag="qq", name="qq0")
    qq1 = qqp.tile([JCH, NQ], F16, tag="qq", name="qq1")
    _emit_qq(nc, pools, wqq_sb, qsc, 0, qq0)

    att = [[None, None], [None, None]]
    for d in range(2):
        for ih in range(NIH):
            att[d][ih] = attp.tile([C, IH], F16, tag="att", name=f"att{d}{ih}")

    # vproj batches interleave with attention passes so the PE never sits
    # behind a full projection block (AV pass c needs only vproj batch c//8,
    # emitted >= 6 passes ahead)
    def vp_aux(feat_sb, m, vt, batches):
        def aux(c):
            if c in batches:
                _emit_vproj_batch(nc, pools, feat_sb, wv_sb, m, vt, batches[c])
        return aux

    def aux00(c):
        vb = {1: 0, 4: 1, 10: 2, 16: 3}
        if c in vb:
            _emit_vproj_batch(nc, pools, fs_mri, wv_sb, 0, vT_mri, vb[c])
        elif c == 24:
            _emit_qq(nc, pools, wqq_sb, qsm, 1, qq1)

    _emit_attention(
        nc, pools, fs_mri, qq0, vT_mri, 0, 0, att[0][0], aux=aux00,
    )
    _emit_attention(
        nc, pools, fs_mri, qq0, vT_mri, 1, 0, att[0][1],
        aux=vp_aux(fs_ct, 1, vT_ct, {2: 0, 10: 1, 18: 2, 26: 3}),
    )
    ot = op_.tile([C, NQ], F32, tag="ot", name="ot")

    def emit_outproj(ih):
        po = pools["sp"].tile([JCH, 2 * IH], F32, tag="ps", name=f"po{ih}")
        nc.tensor.matmul(
            po[0:C, 0:IH], lhsT=woT_sb[:, 0:C], rhs=att[0][ih][:],
            start=True, stop=False,
        )
        nc.tensor.matmul(
            po[0:C, 0:IH], lhsT=woT_sb[:, C : 2 * C], rhs=att[1][ih][:],
            start=False, stop=True,
        )
        nc.vector.tensor_scalar_add(
            ot[:, IH * ih : IH * (ih + 1)], po[0:C, 0:IH], bo_sb[:]
        )

    _emit_attention(nc, pools, fs_ct, qq1, vT_ct, 0, 1, att[1][0])

    # outproj(0) + its store ride inside the last stage (inputs ready ~3us
    # in; the ~1us DMA dispatch overlaps compute) so only outproj(1) and
    # half the store remain in the tail
    def aux11(c):
        if c == 8:
            emit_outproj(0)
            nc.sync.dma_start(out[:, 0:IH], ot[:, 0:IH])

    _emit_attention(nc, pools, fs_ct, qq1, vT_ct, 1, 1, att[1][1], aux=aux11)
    emit_outproj(1)
    nc.sync.dma_start(out[:, IH : 2 * IH], ot[:, IH : 2 * IH])


def build_bass():
    nc = bacc.Bacc("TRN2", target_bir_lowering=False, debug=False)
    ct = nc.dram_tensor("ct_feat", [JCH, HF], F16, kind="ExternalInput").ap()
    mri = nc.dram_tensor("mri_feat", [JCH, HF], F16, kind="ExternalInput").ap()
    smalls = nc.dram_tensor("smalls", [JCH, SMW], F16, kind="ExternalInput").ap()
    out = nc.dram_tensor("out", [C, NQ], F32, kind="ExternalOutput").ap()

    with tile.TileContext(nc) as tc, ExitStack() as ctx:
        _build_program(ctx, tc, ct, mri, smalls, out)
    nc.compile()
    return nc


def _aug(w, b):
    # (out,in) weight + (out,) bias -> [w.T; b] of shape (in+1, out)
    return np.concatenate(
        [np.asarray(w, np.float32).T, np.asarray(b, np.float32)[None, :]], axis=0
    )


def _dup_rows(a):
    return np.concatenate([a, a], axis=0)


def prepare_inputs(inputs):
    scale = np.float32(1.0 / np.sqrt(C))
    ct = np.asarray(inputs["ct_features"], np.float32).reshape(C, N)
    mri = np.asarray(inputs["mri_features"], np.float32).reshape(C, N)
    ones = np.ones((1, N), np.float32)
    # (128, 4096): top half = key chunks 0..31, bottom half = chunks 32..63
    ct_dup = np.concatenate([ct[:, :HF], ct[:, HF:]], axis=0).astype(np.float16)
    mri_dup = np.concatenate([mri[:, :HF], mri[:, HF:]], axis=0).astype(np.float16)
    ct_aug = np.concatenate([ct, ones], axis=0).astype(np.float16)
    mri_aug = np.concatenate([mri, ones], axis=0).astype(np.float16)
    # qq weights: K projection folded onto the query side, K bias dropped
    # (constant per query -> cancels in softmax); scale folded into wq/bq
    wqq = np.concatenate(
        [
            _aug(np.asarray(inputs["wq_ct"]) * scale,
                 np.asarray(inputs["bq_ct"]) * scale)
            @ np.asarray(inputs["wk_mri"], np.float32),
            _aug(np.asarray(inputs["wq_mri"]) * scale,
                 np.asarray(inputs["bq_mri"]) * scale)
            @ np.asarray(inputs["wk_ct"], np.float32),
        ],
        axis=1,
    ).astype(np.float16)
    # V weights (bias folded into bo'), duplicated into both partition halves
    wv_h = np.concatenate(
        [np.asarray(inputs["wv_mri"], np.float32).T,
         np.asarray(inputs["wv_ct"], np.float32).T],
        axis=1,
    )
    wv_dup = _dup_rows(wv_h).astype(np.float16)
    wo = np.asarray(inputs["wo"], np.float32)
    woT = np.concatenate([wo[:, :C].T, wo[:, C:].T], axis=1).astype(np.float16)
    bo2 = (
        np.asarray(inputs["bo"], np.float32)
        + wo[:, :C] @ np.asarray(inputs["bv_mri"], np.float32)
        + wo[:, C:] @ np.asarray(inputs["bv_ct"], np.float32)
    )
    bo2 = bo2[:, None].astype(np.float16)

    in_maps = []
    for i in range(NCORES):
        sl = slice(NQ * i, NQ * (i + 1))
        sm = np.zeros((JCH, SMW), np.float16)
        sm[0:W, 0:NQ] = ct_aug[:, sl]
        sm[0:W, NQ : 2 * NQ] = mri_aug[:, sl]
        sm[0:W, 2 * NQ : 2 * NQ + 2 * C] = wqq
        sm[0:JCH, 2 * NQ + 2 * C : 2 * NQ + 4 * C] = wv_dup
        sm[0:C, 2 * NQ + 4 * C : 2 * NQ + 6 * C] = woT
        sm[0:C, 2 * NQ + 6 * C : 2 * NQ + 6 * C + 1] = bo2
        in_maps.append(
            {
                "ct_feat": ct_dup,
                "mri_feat": mri_dup,
                "smalls": sm,
            }
        )
    return in_maps


def assemble_output(results):
    out = np.concatenate([results[i]["out"] for i in range(NCORES)], axis=1)
    return out.reshape(1, C, 8, 32, 32)


_NC_CACHE = None


def _get_nc():
    global _NC_CACHE
    if _NC_CACHE is None:
        _NC_CACHE = build_bass()
    return _NC_CACHE


def kernel(**inputs):
    nc = _get_nc()
    in_maps = prepare_inputs(inputs)
    res = run_bass_kernel_spmd(nc, in_maps, list(range(NCORES)))
    return assemble_output(res.results)


if __name__ == "__main__":
    nc = build_bass()
    print("built OK")
